# revision 1
# baseline (speedup 1.0000x reference)
"""Distributed Trainium2 kernel for nn_Attention_10857677324470.

Sharding: 8 NeuronCores = batch item b (4) x head-half g (2).
Each core computes, for its (item, head-group-of-4):
  qkv 1x1 conv (768 of 1536 out-channels) -> depthwise 3x3 -> linear
  attention for 4 heads -> crpe refine -> partial output channels.
The only cross-core communication is a pairwise all-gather of the 256-ch
attention output between the two cores sharing one batch item, after which
each core computes its 256 proj output channels.

All weight slicing is done host-side with numpy so the device program is a
plain dense pipeline (no grouped convs except depthwise).
"""

import numpy as np
import jax
import jax.numpy as jnp
from functools import partial

P = 2
HEADS = 8
HG = 2                 # head groups (shards per item)
HPG = HEADS // HG      # heads per group = 4
FF = 4
EPS = 1e-8
WINDOWS = [3, 5, 7]
HEAD_SPLITS = [2, 3, 3]
C = 512
CH = C // HEADS        # 64
H = W = 128
N = H * W
NCORES = 8

_AXIS_GROUPS = [[0, 1], [2, 3], [4, 5], [6, 7]]


def _head_window(h):
    # head -> (crpe index, local index within that crpe's split)
    if h < 2:
        return 0, h
    if h < 5:
        return 1, h - 2
    return 2, h - 5


@partial(jax.pmap, axis_name="x")
def _device_fn(x, qkv_w, dw_w, crpe_w, crpe_b, proj_w, temp, scale):
    """Per-core computation.

    x:      [512, 128, 128]   input image for this core's batch item
    qkv_w:  [768, 512]        rows: q(256) k(256) v(256) for this core's heads
    dw_w:   [768, 3, 3]       depthwise taps for those channels
    crpe_w: [4, 64, 7, 7]     per-head crpe filter (zero-padded to 7x7)
    crpe_b: [4]               per-head crpe bias
    proj_w: [256, 512]        this core's proj output rows (full 512 in-ch)
    temp:   [4, 1, 1]         temperature for this core's heads
    scale:  [4, 1, 1]         scale for this core's heads
    """
    f32 = jnp.float32

    # qkv 1x1 conv as matmul: [768, 512] @ [512, N]
    xf = x.reshape(C, N)
    qkv = (qkv_w @ xf).reshape(1, 768, H, W)

    # depthwise 3x3
    qkv = jax.lax.conv_general_dilated(
        qkv, dw_w[:, None, :, :], window_strides=(1, 1),
        padding=[(1, 1), (1, 1)], feature_group_count=768,
        dimension_numbers=("NCHW", "OIHW", "NCHW"))[0]

    q = qkv[0:256].reshape(HPG, CH, N)
    k = qkv[256:512].reshape(HPG, CH, N)
    v = qkv[512:768].reshape(HPG, CH, N)

    qn = q.transpose(0, 2, 1)                      # h N c
    vn = v.transpose(0, 2, 1)                      # h N c

    q1 = qn / (jnp.linalg.norm(qn, axis=-1, keepdims=True) + EPS)
    k1 = k / (jnp.linalg.norm(k, axis=-2, keepdims=True) + EPS)
    q2 = jax.nn.relu(qn) ** FF
    k2 = jax.nn.relu(k) ** FF
    q2 = q2 / (jnp.linalg.norm(q2, axis=-1, keepdims=True) + EPS)
    k2 = k2 / (jnp.linalg.norm(k2, axis=-2, keepdims=True) + EPS)

    # crpe: per head, 64ch -> 1ch conv (window padded to 7x7)
    v_img = v.reshape(HPG * CH, H, W)[None]
    att = jax.lax.conv_general_dilated(
        v_img, crpe_w, window_strides=(1, 1),
        padding=[(3, 3), (3, 3)], feature_group_count=HPG,
        dimension_numbers=("NCHW", "OIHW", "NCHW"))[0]
    att = att + crpe_b[:, None, None]
    refine = jax.nn.sigmoid(att.reshape(HPG, N, 1))

    # linear attention
    attn1 = jnp.einsum("hcn,hnd->hcd", k1, vn)
    attn2 = jnp.einsum("hcn,hnd->hcd", k2, vn)
    sg = jax.nn.sigmoid(scale)                     # [4,1,1]
    numer = (vn.sum(axis=1, keepdims=True)
             + jnp.einsum("hnc,hcd->hnd", q1, attn1)
             + sg * jnp.einsum("hnc,hcd->hnd", q2, attn2))
    s1 = jnp.einsum("hnc,hc->hn", q1, k1.sum(axis=-1))
    s2 = jnp.einsum("hnc,hc->hn", q2, (sg * k2).sum(axis=-1))
    denom = f32(N) + s1[..., None] + s2[..., None] + EPS

    out = (numer / denom) * temp + refine          # h N c
    out = out.transpose(0, 2, 1).reshape(256, N)   # [256, N]

    # gather the other head-half from the paired core, then proj
    full = jax.lax.all_gather(out, "x", axis_index_groups=_AXIS_GROUPS)
    full = full.reshape(C, N)                      # [512, N]
    o = proj_w @ full                              # [256, N]
    return o.reshape(256, H, W)


def kernel(x, qkv_w, dw_w, proj_w, temperature, scale,
           crpe_w0, crpe_b0, crpe_w1, crpe_b1, crpe_w2, crpe_b2):
    x = np.asarray(x, dtype=np.float32)
    qkv_w = np.asarray(qkv_w, dtype=np.float32).reshape(3072, 512)
    dw_w = np.asarray(dw_w, dtype=np.float32).reshape(3072, 3, 3)
    proj_w = np.asarray(proj_w, dtype=np.float32).reshape(1024, 512)
    temperature = np.asarray(temperature, dtype=np.float32)
    scale = np.asarray(scale, dtype=np.float32)
    crpe_ws = [np.asarray(w, dtype=np.float32) for w in (crpe_w0, crpe_w1, crpe_w2)]
    crpe_bs = [np.asarray(b, dtype=np.float32) for b in (crpe_b0, crpe_b1, crpe_b2)]

    b = x.shape[0]
    B = b // P

    xs, qkvs, dws, crpews, crpebs, projs, temps, scales = ([] for _ in range(8))
    for core in range(NCORES):
        item = core // HG
        g = core % HG
        p = item // B                      # path of this batch item
        heads = list(range(g * HPG, (g + 1) * HPG))

        # qkv rows for path p: q block then k,v; within each, this group's heads
        base = p * 3 * C
        rows = []
        for sec in range(3):               # q, k, v sections
            lo = base + sec * C + g * HPG * CH
            rows.append(np.arange(lo, lo + HPG * CH))
        rows = np.concatenate(rows)
        qkvs.append(qkv_w[rows])
        dws.append(dw_w[rows])

        # crpe filters: pad every window to 7x7 (zero padding keeps conv exact)
        cw = np.zeros((HPG, CH, 7, 7), dtype=np.float32)
        cb = np.zeros((HPG,), dtype=np.float32)
        for j, h in enumerate(heads):
            wi, li = _head_window(h)
            hs = HEAD_SPLITS[wi]
            win = WINDOWS[wi]
            pad = (7 - win) // 2
            cw[j, :, pad:7 - pad, pad:7 - pad] = crpe_ws[wi][p * hs + li]
            cb[j] = crpe_bs[wi][p * hs + li]
        crpews.append(cw)
        crpebs.append(cb)

        projs.append(proj_w[p * C + g * 256: p * C + (g + 1) * 256])
        temps.append(temperature[p, heads])
        scales.append(scale[p, heads])
        xs.append(x[item])

    args = [np.stack(a) for a in (xs, qkvs, dws, crpews, crpebs, projs, temps, scales)]
    outs = np.asarray(_device_fn(*args))   # [8, 256, 128, 128]

    result = np.empty((b, C, H, W), dtype=np.float32)
    for core in range(NCORES):
        item, g = core // HG, core % HG
        result[item, g * 256:(g + 1) * 256] = outs[core]
    return result


# revision 5
# speedup vs baseline: 203.7983x; 203.7983x over previous
"""Distributed Trainium2 kernel for nn_Attention_10857677324470.

Sharding: 8 NeuronCores = batch item b (4) x head-half g (2).
Each core computes, for its (item, head-group-of-4):
  qkv 1x1 conv (768 of 1536 out-channels) -> depthwise 3x3 -> linear
  attention for 4 heads -> crpe refine -> partial output channels.
The only cross-core communication is a pairwise all-gather of the 256-ch
attention output between the two cores sharing one batch item, after which
each core computes its 256 proj output channels.

All weight slicing is done host-side with numpy so the device program is a
plain dense pipeline (no grouped convs except depthwise).
"""

import numpy as np
import jax
import jax.numpy as jnp
from functools import partial

P = 2
HEADS = 8
HG = 2                 # head groups (shards per item)
HPG = HEADS // HG      # heads per group = 4
FF = 4
EPS = 1e-8
WINDOWS = [3, 5, 7]
HEAD_SPLITS = [2, 3, 3]
C = 512
CH = C // HEADS        # 64
H = W = 128
N = H * W
NCORES = 8

_AXIS_GROUPS = [[0, 1], [2, 3], [4, 5], [6, 7]]


def _head_window(h):
    # head -> (crpe index, local index within that crpe's split)
    if h < 2:
        return 0, h
    if h < 5:
        return 1, h - 2
    return 2, h - 5


def _dw3x3(t, w):
    """Depthwise 3x3 as 9 shifted multiply-adds. t: [C', H, W], w: [C', 3, 3]."""
    tp = jnp.pad(t, ((0, 0), (1, 1), (1, 1)))
    out = None
    for dy in range(3):
        for dx in range(3):
            term = w[:, dy, dx][:, None, None] * \
                jax.lax.dynamic_slice(tp, (0, dy, dx), t.shape)
            out = term if out is None else out + term
    return out


@partial(jax.pmap, axis_name="x")
def _device_fn(x, qkv_w, dw_w, crpe_w, crpe_b, proj_w, temp, scale):
    """Per-core computation.

    x:      [512, 128, 128]   input image for this core's batch item
    qkv_w:  [768, 512]        rows: q(256) k(256) v(256) for this core's heads
    dw_w:   [768, 3, 3]       depthwise taps for those channels
    crpe_w: [4, 64, 7, 7]     per-head crpe filter (zero-padded to 7x7)
    crpe_b: [4]               per-head crpe bias
    proj_w: [256, 512]        this core's proj output rows (full 512 in-ch)
    temp:   [4, 1, 1]         temperature for this core's heads
    scale:  [4, 1, 1]         scale for this core's heads
    """
    f32 = jnp.float32
    bf16 = jnp.bfloat16

    # qkv 1x1 conv as bf16 matmul with fp32 accumulation
    xf = x.reshape(C, N).astype(bf16)
    qkv = jax.lax.dot(qkv_w.astype(bf16), xf,
                      preferred_element_type=f32)       # [768, N]
    qkv = _dw3x3(qkv.reshape(768, H, W), dw_w).reshape(768, N)

    q = qkv[0:256].reshape(HPG, CH, N)
    k = qkv[256:512].reshape(HPG, CH, N)
    v = qkv[512:768].reshape(HPG, CH, N)

    # per-pixel normalizers (norm over channel axis)
    q1s = 1.0 / (jnp.sqrt((q * q).sum(axis=1, keepdims=True)) + EPS)
    k1s = 1.0 / (jnp.sqrt((k * k).sum(axis=1, keepdims=True)) + EPS)
    qr = jax.nn.relu(q)
    kr = jax.nn.relu(k)
    qr2 = qr * qr
    kr2 = kr * kr
    q4 = qr2 * qr2                                      # relu(q)^4
    k4 = kr2 * kr2
    q2s = 1.0 / (jnp.sqrt((q4 * q4).sum(axis=1, keepdims=True)) + EPS)
    k2s = 1.0 / (jnp.sqrt((k4 * k4).sum(axis=1, keepdims=True)) + EPS)

    q1 = (q * q1s).astype(bf16)                         # h c n
    k1 = (k * k1s).astype(bf16)
    q2 = (q4 * q2s).astype(bf16)
    k2 = (k4 * k2s).astype(bf16)
    vb = v.astype(bf16)

    # crpe: per head, 64ch -> 7x7 -> 1ch  (tap sum of small contractions)
    vp = jnp.pad(v.reshape(HPG, CH, H, W), ((0, 0), (0, 0), (3, 3), (3, 3)))
    att = crpe_b[:, None, None]
    for dy in range(7):
        for dx in range(7):
            sl = jax.lax.dynamic_slice(vp, (0, 0, dy, dx), (HPG, CH, H, W))
            att = att + jnp.einsum("hc,hcyx->hyx", crpe_w[:, :, dy, dx], sl)
    refine = jax.nn.sigmoid(att.reshape(HPG, 1, N))     # h 1 n

    # linear attention (channel-major throughout; contraction over pixels)
    attn1 = jax.lax.dot_general(k1, vb, (((2,), (2,)), ((0,), (0,))),
                                preferred_element_type=f32)   # h c d
    attn2 = jax.lax.dot_general(k2, vb, (((2,), (2,)), ((0,), (0,))),
                                preferred_element_type=f32)
    sg = jax.nn.sigmoid(scale)                          # [4,1,1]
    vsum = v.sum(axis=2, keepdims=True)                 # h d 1
    m1 = jax.lax.dot_general(attn1.astype(bf16), q1,
                             (((1,), (1,)), ((0,), (0,))),
                             preferred_element_type=f32)      # h d n
    m2 = jax.lax.dot_general((sg * attn2).astype(bf16), q2,
                             (((1,), (1,)), ((0,), (0,))),
                             preferred_element_type=f32)
    numer = vsum + m1 + m2                              # h d n
    k1sum = k1.astype(f32).sum(axis=-1)                 # h c (f32 accumulate)
    k2sum = k2.astype(f32).sum(axis=-1) * sg[:, :, 0]   # h c, scale folded in
    s1 = jnp.einsum("hcn,hc->hn", q1, k1sum.astype(bf16),
                    preferred_element_type=f32)
    s2 = jnp.einsum("hcn,hc->hn", q2, k2sum.astype(bf16),
                    preferred_element_type=f32)
    denom = f32(N) + s1[:, None, :] + s2[:, None, :] + EPS

    out = (numer / denom) * temp + refine               # h d n
    out = out.reshape(256, N)

    # gather the other head-half from the paired core, then proj
    full = jax.lax.all_gather(out.astype(bf16), "x",
                              axis_index_groups=_AXIS_GROUPS)
    full = full.reshape(C, N)                           # [512, N]
    o = jax.lax.dot(proj_w.astype(bf16), full,
                    preferred_element_type=f32)         # [256, N]
    return o.reshape(256, H, W)


def _build_args(x, qkv_w, dw_w, proj_w, temperature, scale,
                crpe_w0, crpe_b0, crpe_w1, crpe_b1, crpe_w2, crpe_b2):
    x = np.asarray(x, dtype=np.float32)
    qkv_w = np.asarray(qkv_w, dtype=np.float32).reshape(3072, 512)
    dw_w = np.asarray(dw_w, dtype=np.float32).reshape(3072, 3, 3)
    proj_w = np.asarray(proj_w, dtype=np.float32).reshape(1024, 512)
    temperature = np.asarray(temperature, dtype=np.float32)
    scale = np.asarray(scale, dtype=np.float32)
    crpe_ws = [np.asarray(w, dtype=np.float32) for w in (crpe_w0, crpe_w1, crpe_w2)]
    crpe_bs = [np.asarray(b, dtype=np.float32) for b in (crpe_b0, crpe_b1, crpe_b2)]

    b = x.shape[0]
    B = b // P

    xs, qkvs, dws, crpews, crpebs, projs, temps, scales = ([] for _ in range(8))
    for core in range(NCORES):
        item = core // HG
        g = core % HG
        p = item // B                      # path of this batch item
        heads = list(range(g * HPG, (g + 1) * HPG))

        # qkv rows for path p: q block then k,v; within each, this group's heads
        base = p * 3 * C
        rows = []
        for sec in range(3):               # q, k, v sections
            lo = base + sec * C + g * HPG * CH
            rows.append(np.arange(lo, lo + HPG * CH))
        rows = np.concatenate(rows)
        qkvs.append(qkv_w[rows])
        dws.append(dw_w[rows])

        # crpe filters: pad every window to 7x7 (zero padding keeps conv exact)
        cw = np.zeros((HPG, CH, 7, 7), dtype=np.float32)
        cb = np.zeros((HPG,), dtype=np.float32)
        for j, h in enumerate(heads):
            wi, li = _head_window(h)
            hs = HEAD_SPLITS[wi]
            win = WINDOWS[wi]
            pad = (7 - win) // 2
            cw[j, :, pad:7 - pad, pad:7 - pad] = crpe_ws[wi][p * hs + li]
            cb[j] = crpe_bs[wi][p * hs + li]
        crpews.append(cw)
        crpebs.append(cb)

        projs.append(proj_w[p * C + g * 256: p * C + (g + 1) * 256])
        temps.append(temperature[p, heads])
        scales.append(scale[p, heads])
        xs.append(x[item])

    return [np.stack(a) for a in (xs, qkvs, dws, crpews, crpebs, projs, temps, scales)], b


def _assemble(outs, b):
    result = np.empty((b, C, H, W), dtype=np.float32)
    for core in range(NCORES):
        item, g = core // HG, core % HG
        result[item, g * 256:(g + 1) * 256] = outs[core]
    return result


def kernel(**inputs):
    args, b = _build_args(**inputs)
    outs = np.asarray(_device_fn(*args))   # [8, 256, 128, 128]
    return _assemble(outs, b)


# revision 6
# speedup vs baseline: 270.5486x; 1.3275x over previous
"""Distributed Trainium2 kernel for nn_Attention_10857677324470.

Sharding: 8 NeuronCores = batch item b (4) x head-half g (2).
Each core computes, for its (item, head-group-of-4):
  qkv 1x1 conv (768 of 1536 out-channels) -> depthwise 3x3 -> linear
  attention for 4 heads -> crpe refine -> partial output channels.
The only cross-core communication is a pairwise all-gather of the 256-ch
attention output between the two cores sharing one batch item, after which
each core computes its 256 proj output channels.

All weight slicing is done host-side with numpy so the device program is a
plain dense pipeline (no grouped convs except depthwise).
"""

import numpy as np
import jax
import jax.numpy as jnp
from functools import partial

P = 2
HEADS = 8
HG = 2                 # head groups (shards per item)
HPG = HEADS // HG      # heads per group = 4
FF = 4
EPS = 1e-8
WINDOWS = [3, 5, 7]
HEAD_SPLITS = [2, 3, 3]
C = 512
CH = C // HEADS        # 64
H = W = 128
N = H * W
NCORES = 8

_AXIS_GROUPS = [[0, 1], [2, 3], [4, 5], [6, 7]]


def _head_window(h):
    # head -> (crpe index, local index within that crpe's split)
    if h < 2:
        return 0, h
    if h < 5:
        return 1, h - 2
    return 2, h - 5


def _dw3x3(t, w):
    """Depthwise 3x3 as 9 shifted multiply-adds. t: [C', H, W], w: [C', 3, 3]."""
    tp = jnp.pad(t, ((0, 0), (1, 1), (1, 1)))
    out = None
    for dy in range(3):
        for dx in range(3):
            term = w[:, dy, dx][:, None, None] * \
                jax.lax.dynamic_slice(tp, (0, dy, dx), t.shape)
            out = term if out is None else out + term
    return out


@partial(jax.pmap, axis_name="x")
def _device_fn(x, qkv_w, dw_w, crpe_w, crpe_b, proj_w, temp, scale):
    """Per-core computation.

    x:      [512, 128, 128]   input image for this core's batch item
    qkv_w:  [768, 512]        rows: q(256) k(256) v(256) for this core's heads
    dw_w:   [768, 3, 3]       depthwise taps for those channels
    crpe_w: [4, 64, 7, 7]     per-head crpe filter (zero-padded to 7x7)
    crpe_b: [4]               per-head crpe bias
    proj_w: [256, 512]        this core's proj output rows (full 512 in-ch)
    temp:   [4, 1, 1]         temperature for this core's heads
    scale:  [4, 1, 1]         scale for this core's heads
    """
    f32 = jnp.float32
    bf16 = jnp.bfloat16

    # qkv 1x1 conv as bf16 matmul with fp32 accumulation
    xf = x.reshape(C, N).astype(bf16)
    qkv = jax.lax.dot(qkv_w.astype(bf16), xf,
                      preferred_element_type=f32)       # [768, N]
    qkv = _dw3x3(qkv.reshape(768, H, W), dw_w).reshape(768, N)

    q = qkv[0:256].reshape(HPG, CH, N)
    k = qkv[256:512].reshape(HPG, CH, N)
    v = qkv[512:768].reshape(HPG, CH, N)

    # per-pixel normalizers (norm over channel axis)
    q1s = 1.0 / (jnp.sqrt((q * q).sum(axis=1, keepdims=True)) + EPS)
    k1s = 1.0 / (jnp.sqrt((k * k).sum(axis=1, keepdims=True)) + EPS)
    qr = jax.nn.relu(q)
    kr = jax.nn.relu(k)
    qr2 = qr * qr
    kr2 = kr * kr
    q4 = qr2 * qr2                                      # relu(q)^4
    k4 = kr2 * kr2
    q2s = 1.0 / (jnp.sqrt((q4 * q4).sum(axis=1, keepdims=True)) + EPS)
    k2s = 1.0 / (jnp.sqrt((k4 * k4).sum(axis=1, keepdims=True)) + EPS)

    q1 = (q * q1s).astype(bf16)                         # h c n
    k1 = (k * k1s).astype(bf16)
    q2 = (q4 * q2s).astype(bf16)
    k2 = (k4 * k2s).astype(bf16)
    vb = v.astype(bf16)

    # crpe: per head, 64ch -> 7x7 -> 1ch  (grouped conv, 4 groups)
    att = jax.lax.conv_general_dilated(
        v.reshape(1, HPG * CH, H, W), crpe_w, (1, 1), [(3, 3), (3, 3)],
        feature_group_count=HPG,
        dimension_numbers=("NCHW", "OIHW", "NCHW"))[0]
    refine = jax.nn.sigmoid(
        (att + crpe_b[:, None, None]).reshape(HPG, 1, N))   # h 1 n

    # linear attention (channel-major throughout; contraction over pixels)
    attn1 = jax.lax.dot_general(k1, vb, (((2,), (2,)), ((0,), (0,))),
                                preferred_element_type=f32)   # h c d
    attn2 = jax.lax.dot_general(k2, vb, (((2,), (2,)), ((0,), (0,))),
                                preferred_element_type=f32)
    sg = jax.nn.sigmoid(scale)                          # [4,1,1]
    vsum = v.sum(axis=2, keepdims=True)                 # h d 1
    m1 = jax.lax.dot_general(attn1.astype(bf16), q1,
                             (((1,), (1,)), ((0,), (0,))),
                             preferred_element_type=f32)      # h d n
    m2 = jax.lax.dot_general((sg * attn2).astype(bf16), q2,
                             (((1,), (1,)), ((0,), (0,))),
                             preferred_element_type=f32)
    numer = vsum + m1 + m2                              # h d n
    k1sum = k1.astype(f32).sum(axis=-1)                 # h c (f32 accumulate)
    k2sum = k2.astype(f32).sum(axis=-1) * sg[:, :, 0]   # h c, scale folded in
    s1 = jnp.einsum("hcn,hc->hn", q1, k1sum.astype(bf16),
                    preferred_element_type=f32)
    s2 = jnp.einsum("hcn,hc->hn", q2, k2sum.astype(bf16),
                    preferred_element_type=f32)
    denom = f32(N) + s1[:, None, :] + s2[:, None, :] + EPS

    out = (numer / denom) * temp + refine               # h d n
    out = out.reshape(256, N)

    # gather the other head-half from the paired core, then proj
    full = jax.lax.all_gather(out.astype(bf16), "x",
                              axis_index_groups=_AXIS_GROUPS)
    full = full.reshape(C, N)                           # [512, N]
    o = jax.lax.dot(proj_w.astype(bf16), full,
                    preferred_element_type=f32)         # [256, N]
    return o.reshape(256, H, W)


def _build_args(x, qkv_w, dw_w, proj_w, temperature, scale,
                crpe_w0, crpe_b0, crpe_w1, crpe_b1, crpe_w2, crpe_b2):
    x = np.asarray(x, dtype=np.float32)
    qkv_w = np.asarray(qkv_w, dtype=np.float32).reshape(3072, 512)
    dw_w = np.asarray(dw_w, dtype=np.float32).reshape(3072, 3, 3)
    proj_w = np.asarray(proj_w, dtype=np.float32).reshape(1024, 512)
    temperature = np.asarray(temperature, dtype=np.float32)
    scale = np.asarray(scale, dtype=np.float32)
    crpe_ws = [np.asarray(w, dtype=np.float32) for w in (crpe_w0, crpe_w1, crpe_w2)]
    crpe_bs = [np.asarray(b, dtype=np.float32) for b in (crpe_b0, crpe_b1, crpe_b2)]

    b = x.shape[0]
    B = b // P

    xs, qkvs, dws, crpews, crpebs, projs, temps, scales = ([] for _ in range(8))
    for core in range(NCORES):
        item = core // HG
        g = core % HG
        p = item // B                      # path of this batch item
        heads = list(range(g * HPG, (g + 1) * HPG))

        # qkv rows for path p: q block then k,v; within each, this group's heads
        base = p * 3 * C
        rows = []
        for sec in range(3):               # q, k, v sections
            lo = base + sec * C + g * HPG * CH
            rows.append(np.arange(lo, lo + HPG * CH))
        rows = np.concatenate(rows)
        qkvs.append(qkv_w[rows])
        dws.append(dw_w[rows])

        # crpe filters: pad every window to 7x7 (zero padding keeps conv exact)
        cw = np.zeros((HPG, CH, 7, 7), dtype=np.float32)
        cb = np.zeros((HPG,), dtype=np.float32)
        for j, h in enumerate(heads):
            wi, li = _head_window(h)
            hs = HEAD_SPLITS[wi]
            win = WINDOWS[wi]
            pad = (7 - win) // 2
            cw[j, :, pad:7 - pad, pad:7 - pad] = crpe_ws[wi][p * hs + li]
            cb[j] = crpe_bs[wi][p * hs + li]
        crpews.append(cw)
        crpebs.append(cb)

        projs.append(proj_w[p * C + g * 256: p * C + (g + 1) * 256])
        temps.append(temperature[p, heads])
        scales.append(scale[p, heads])
        xs.append(x[item])

    return [np.stack(a) for a in (xs, qkvs, dws, crpews, crpebs, projs, temps, scales)], b


def _assemble(outs, b):
    result = np.empty((b, C, H, W), dtype=np.float32)
    for core in range(NCORES):
        item, g = core // HG, core % HG
        result[item, g * 256:(g + 1) * 256] = outs[core]
    return result


def kernel(**inputs):
    args, b = _build_args(**inputs)
    outs = np.asarray(_device_fn(*args))   # [8, 256, 128, 128]
    return _assemble(outs, b)


# revision 9
# speedup vs baseline: 273.4543x; 1.0107x over previous
"""Distributed Trainium2 kernel for nn_Attention_10857677324470.

Sharding: 8 NeuronCores = batch item b (4) x head-half g (2).
Each core computes, for its (item, head-group-of-4):
  qkv 1x1 conv (768 of 1536 out-channels) -> depthwise 3x3 -> linear
  attention for 4 heads -> crpe refine -> partial output channels.
The only cross-core communication is a pairwise all-gather of the 256-ch
attention output between the two cores sharing one batch item, after which
each core computes its 256 proj output channels.

All weight slicing is done host-side with numpy so the device program is a
plain dense pipeline (no grouped convs except depthwise).
"""

import numpy as np
import jax
import jax.numpy as jnp
from functools import partial

P = 2
HEADS = 8
HG = 2                 # head groups (shards per item)
HPG = HEADS // HG      # heads per group = 4
FF = 4
EPS = 1e-8
WINDOWS = [3, 5, 7]
HEAD_SPLITS = [2, 3, 3]
C = 512
CH = C // HEADS        # 64
H = W = 128
N = H * W
NCORES = 8

_AXIS_GROUPS = [[0, 1], [2, 3], [4, 5], [6, 7]]


def _head_window(h):
    # head -> (crpe index, local index within that crpe's split)
    if h < 2:
        return 0, h
    if h < 5:
        return 1, h - 2
    return 2, h - 5


def _dw3x3(t, w):
    """Depthwise 3x3 as 9 shifted multiply-adds.

    t: [C', H, W] (bf16 reads, f32 accumulation via the f32 weights),
    w: [C', 3, 3] f32.
    """
    tp = jnp.pad(t.astype(jnp.bfloat16), ((0, 0), (1, 1), (1, 1)))
    out = None
    for dy in range(3):
        for dx in range(3):
            term = w[:, dy, dx][:, None, None] * \
                jax.lax.dynamic_slice(tp, (0, dy, dx), tp.shape[:1] + (H, W))
            out = term if out is None else out + term
    return out


@partial(jax.pmap, axis_name="x")
def _device_fn(x, qkv_w, dw_w, crpe_w, crpe_b, proj_w, temp, scale):
    """Per-core computation.

    x:      [512, 128, 128]   input image for this core's batch item
    qkv_w:  [768, 512]        rows: q(256) k(256) v(256) for this core's heads
    dw_w:   [768, 3, 3]       depthwise taps for those channels
    crpe_w: [4, 64, 7, 7]     per-head crpe filter (zero-padded to 7x7)
    crpe_b: [4]               per-head crpe bias
    proj_w: [256, 512]        this core's proj output rows (full 512 in-ch)
    temp:   [4, 1, 1]         temperature for this core's heads
    scale:  [4, 1, 1]         scale for this core's heads
    """
    f32 = jnp.float32
    bf16 = jnp.bfloat16

    # qkv 1x1 conv as bf16 matmul with fp32 accumulation (x arrives bf16)
    xf = x.reshape(C, N)
    qkv = jax.lax.dot(qkv_w.astype(bf16), xf,
                      preferred_element_type=f32)       # [768, N]
    qkv = _dw3x3(qkv.reshape(768, H, W), dw_w).reshape(768, N)

    q = qkv[0:256].reshape(HPG, CH, N)
    k = qkv[256:512].reshape(HPG, CH, N)
    v = qkv[512:768].reshape(HPG, CH, N)

    # per-pixel normalizers (norm over channel axis)
    q1s = 1.0 / (jnp.sqrt((q * q).sum(axis=1, keepdims=True)) + EPS)
    k1s = 1.0 / (jnp.sqrt((k * k).sum(axis=1, keepdims=True)) + EPS)
    qr = jax.nn.relu(q)
    kr = jax.nn.relu(k)
    qr2 = qr * qr
    kr2 = kr * kr
    q4 = qr2 * qr2                                      # relu(q)^4
    k4 = kr2 * kr2
    q2s = 1.0 / (jnp.sqrt((q4 * q4).sum(axis=1, keepdims=True)) + EPS)
    k2s = 1.0 / (jnp.sqrt((k4 * k4).sum(axis=1, keepdims=True)) + EPS)

    q1 = (q * q1s).astype(bf16)                         # h c n
    k1 = (k * k1s).astype(bf16)
    q2 = (q4 * q2s).astype(bf16)
    k2 = (k4 * k2s).astype(bf16)
    vb = v.astype(bf16)

    # crpe: per head, 64ch -> 7x7 -> 1ch  (grouped conv, 4 groups)
    att = jax.lax.conv_general_dilated(
        v.reshape(1, HPG * CH, H, W), crpe_w, (1, 1), [(3, 3), (3, 3)],
        feature_group_count=HPG,
        dimension_numbers=("NCHW", "OIHW", "NCHW"))[0]
    refine = jax.nn.sigmoid(
        (att + crpe_b[:, None, None]).reshape(HPG, 1, N))   # h 1 n

    # linear attention (channel-major throughout; contraction over pixels)
    attn1 = jax.lax.dot_general(k1, vb, (((2,), (2,)), ((0,), (0,))),
                                preferred_element_type=f32)   # h c d
    attn2 = jax.lax.dot_general(k2, vb, (((2,), (2,)), ((0,), (0,))),
                                preferred_element_type=f32)
    sg = jax.nn.sigmoid(scale)                          # [4,1,1]
    vsum = v.sum(axis=2, keepdims=True)                 # h d 1
    m1 = jax.lax.dot_general(attn1.astype(bf16), q1,
                             (((1,), (1,)), ((0,), (0,))),
                             preferred_element_type=f32)      # h d n
    m2 = jax.lax.dot_general((sg * attn2).astype(bf16), q2,
                             (((1,), (1,)), ((0,), (0,))),
                             preferred_element_type=f32)
    numer = vsum + m1 + m2                              # h d n
    k1sum = k1.astype(f32).sum(axis=-1)                 # h c (f32 accumulate)
    k2sum = k2.astype(f32).sum(axis=-1) * sg[:, :, 0]   # h c, scale folded in
    s1 = jnp.einsum("hcn,hc->hn", q1, k1sum.astype(bf16),
                    preferred_element_type=f32)
    s2 = jnp.einsum("hcn,hc->hn", q2, k2sum.astype(bf16),
                    preferred_element_type=f32)
    denom = f32(N) + s1[:, None, :] + s2[:, None, :] + EPS

    out = (numer / denom) * temp + refine               # h d n
    out = out.reshape(256, N)

    # gather the other head-half from the paired core, then proj
    full = jax.lax.all_gather(out.astype(bf16), "x",
                              axis_index_groups=_AXIS_GROUPS)
    full = full.reshape(C, N)                           # [512, N]
    o = jax.lax.dot(proj_w.astype(bf16), full,
                    preferred_element_type=f32)         # [256, N]
    return o.reshape(256, H, W)


def _build_args(x, qkv_w, dw_w, proj_w, temperature, scale,
                crpe_w0, crpe_b0, crpe_w1, crpe_b1, crpe_w2, crpe_b2):
    x = np.asarray(x, dtype=np.float32)
    qkv_w = np.asarray(qkv_w, dtype=np.float32).reshape(3072, 512)
    dw_w = np.asarray(dw_w, dtype=np.float32).reshape(3072, 3, 3)
    proj_w = np.asarray(proj_w, dtype=np.float32).reshape(1024, 512)
    temperature = np.asarray(temperature, dtype=np.float32)
    scale = np.asarray(scale, dtype=np.float32)
    crpe_ws = [np.asarray(w, dtype=np.float32) for w in (crpe_w0, crpe_w1, crpe_w2)]
    crpe_bs = [np.asarray(b, dtype=np.float32) for b in (crpe_b0, crpe_b1, crpe_b2)]

    b = x.shape[0]
    B = b // P

    xs, qkvs, dws, crpews, crpebs, projs, temps, scales = ([] for _ in range(8))
    for core in range(NCORES):
        item = core // HG
        g = core % HG
        p = item // B                      # path of this batch item
        heads = list(range(g * HPG, (g + 1) * HPG))

        # qkv rows for path p: q block then k,v; within each, this group's heads
        base = p * 3 * C
        rows = []
        for sec in range(3):               # q, k, v sections
            lo = base + sec * C + g * HPG * CH
            rows.append(np.arange(lo, lo + HPG * CH))
        rows = np.concatenate(rows)
        qkvs.append(qkv_w[rows])
        dws.append(dw_w[rows])

        # crpe filters: pad every window to 7x7 (zero padding keeps conv exact)
        cw = np.zeros((HPG, CH, 7, 7), dtype=np.float32)
        cb = np.zeros((HPG,), dtype=np.float32)
        for j, h in enumerate(heads):
            wi, li = _head_window(h)
            hs = HEAD_SPLITS[wi]
            win = WINDOWS[wi]
            pad = (7 - win) // 2
            cw[j, :, pad:7 - pad, pad:7 - pad] = crpe_ws[wi][p * hs + li]
            cb[j] = crpe_bs[wi][p * hs + li]
        crpews.append(cw)
        crpebs.append(cb)

        projs.append(proj_w[p * C + g * 256: p * C + (g + 1) * 256])
        temps.append(temperature[p, heads])
        scales.append(scale[p, heads])
        # ship x as bf16: the device casts to bf16 before the qkv matmul
        # anyway, so this halves the dominant transfer at no numerical cost
        xs.append(x[item].astype(jnp.bfloat16))

    return [np.stack(a) for a in (xs, qkvs, dws, crpews, crpebs, projs, temps, scales)], b


def _assemble(outs, b):
    result = np.empty((b, C, H, W), dtype=np.float32)
    for core in range(NCORES):
        item, g = core // HG, core % HG
        result[item, g * 256:(g + 1) * 256] = outs[core]
    return result


def kernel(**inputs):
    args, b = _build_args(**inputs)
    outs = np.asarray(_device_fn(*args))   # [8, 256, 128, 128]
    return _assemble(outs, b)


# revision 11
# speedup vs baseline: 301.7916x; 1.1036x over previous
"""Distributed Trainium2 kernel for nn_Attention_10857677324470.

Sharding: 8 NeuronCores = batch item b (4) x head-half g (2).
Each core computes, for its (item, head-group-of-4):
  qkv 1x1 conv (768 of 1536 out-channels) -> depthwise 3x3 -> linear
  attention for 4 heads -> crpe refine -> partial output channels.
The only cross-core communication is a pairwise all-gather of the 256-ch
attention output between the two cores sharing one batch item, after which
each core computes its 256 proj output channels.

All weight slicing is done host-side with numpy so the device program is a
plain dense pipeline (no grouped convs except depthwise).
"""

import numpy as np
import jax
import jax.numpy as jnp
from functools import partial

P = 2
HEADS = 8
HG = 2                 # head groups (shards per item)
HPG = HEADS // HG      # heads per group = 4
FF = 4
EPS = 1e-8
WINDOWS = [3, 5, 7]
HEAD_SPLITS = [2, 3, 3]
C = 512
CH = C // HEADS        # 64
H = W = 128
N = H * W
NCORES = 8

_AXIS_GROUPS = [[0, 1], [2, 3], [4, 5], [6, 7]]


def _head_window(h):
    # head -> (crpe index, local index within that crpe's split)
    if h < 2:
        return 0, h
    if h < 5:
        return 1, h - 2
    return 2, h - 5


def _dw3x3(t, w):
    """Depthwise 3x3 as 9 shifted multiply-adds.

    t: [C', H, W] (bf16 reads, f32 accumulation via the f32 weights),
    w: [C', 3, 3] f32.
    """
    tp = jnp.pad(t.astype(jnp.bfloat16), ((0, 0), (1, 1), (1, 1)))
    out = None
    for dy in range(3):
        for dx in range(3):
            term = w[:, dy, dx][:, None, None] * \
                jax.lax.dynamic_slice(tp, (0, dy, dx), tp.shape[:1] + (H, W))
            out = term if out is None else out + term
    return out


@partial(jax.pmap, axis_name="x")
def _device_fn(x, qkv_w, dw_w, crpe_w, crpe_b, proj_w, temp, scale):
    """Per-core computation.

    x:      [512, 128, 128]   input image for this core's batch item
    qkv_w:  [768, 512]        rows: q(256) k(256) v(256) for this core's heads
    dw_w:   [768, 3, 3]       depthwise taps for those channels
    crpe_w: [4, 64, 7, 7]     per-head crpe filter (zero-padded to 7x7)
    crpe_b: [4]               per-head crpe bias
    proj_w: [256, 512]        this core's proj output rows (full 512 in-ch)
    temp:   [4, 1, 1]         temperature for this core's heads
    scale:  [4, 1, 1]         scale for this core's heads
    """
    f32 = jnp.float32
    bf16 = jnp.bfloat16

    # qkv 1x1 conv as bf16 matmul with fp32 accumulation (x arrives bf16)
    xf = x.reshape(C, N)
    qkv = jax.lax.dot(qkv_w.astype(bf16), xf,
                      preferred_element_type=f32)       # [768, N]
    qkv = _dw3x3(qkv.reshape(768, H, W), dw_w).reshape(768, N)

    q = qkv[0:256].reshape(HPG, CH, N)
    k = qkv[256:512].reshape(HPG, CH, N)
    v = qkv[512:768].reshape(HPG, CH, N)

    # per-pixel normalizers (norm over channel axis)
    q1s = 1.0 / (jnp.sqrt((q * q).sum(axis=1, keepdims=True)) + EPS)
    k1s = 1.0 / (jnp.sqrt((k * k).sum(axis=1, keepdims=True)) + EPS)
    qr = jax.nn.relu(q)
    kr = jax.nn.relu(k)
    qr2 = qr * qr
    kr2 = kr * kr
    q4 = qr2 * qr2                                      # relu(q)^4
    k4 = kr2 * kr2
    q2s = 1.0 / (jnp.sqrt((q4 * q4).sum(axis=1, keepdims=True)) + EPS)
    k2s = 1.0 / (jnp.sqrt((k4 * k4).sum(axis=1, keepdims=True)) + EPS)

    q1 = (q * q1s).astype(bf16)                         # h c n
    k1 = (k * k1s).astype(bf16)
    q2 = (q4 * q2s).astype(bf16)
    k2 = (k4 * k2s).astype(bf16)
    vb = v.astype(bf16)

    # crpe: per head, 64ch -> 7x7 -> 1ch  (grouped conv, 4 groups)
    att = jax.lax.conv_general_dilated(
        v.reshape(1, HPG * CH, H, W), crpe_w, (1, 1), [(3, 3), (3, 3)],
        feature_group_count=HPG,
        dimension_numbers=("NCHW", "OIHW", "NCHW"))[0]
    refine = jax.nn.sigmoid(
        (att + crpe_b[:, None, None]).reshape(HPG, 1, N))   # h 1 n

    # linear attention (channel-major throughout; contraction over pixels)
    attn1 = jax.lax.dot_general(k1, vb, (((2,), (2,)), ((0,), (0,))),
                                preferred_element_type=f32)   # h c d
    attn2 = jax.lax.dot_general(k2, vb, (((2,), (2,)), ((0,), (0,))),
                                preferred_element_type=f32)
    sg = jax.nn.sigmoid(scale)                          # [4,1,1]
    vsum = v.sum(axis=2, keepdims=True)                 # h d 1
    m1 = jax.lax.dot_general(attn1.astype(bf16), q1,
                             (((1,), (1,)), ((0,), (0,))),
                             preferred_element_type=f32)      # h d n
    m2 = jax.lax.dot_general((sg * attn2).astype(bf16), q2,
                             (((1,), (1,)), ((0,), (0,))),
                             preferred_element_type=f32)
    numer = vsum + m1 + m2                              # h d n
    k1sum = k1.astype(f32).sum(axis=-1)                 # h c (f32 accumulate)
    k2sum = k2.astype(f32).sum(axis=-1) * sg[:, :, 0]   # h c, scale folded in
    s1 = jnp.einsum("hcn,hc->hn", q1, k1sum.astype(bf16),
                    preferred_element_type=f32)
    s2 = jnp.einsum("hcn,hc->hn", q2, k2sum.astype(bf16),
                    preferred_element_type=f32)
    denom = f32(N) + s1[:, None, :] + s2[:, None, :] + EPS

    out = (numer / denom) * temp + refine               # h d n
    out = out.reshape(256, N)

    # gather the other head-half from the paired core, then proj
    full = jax.lax.all_gather(out.astype(bf16), "x",
                              axis_index_groups=_AXIS_GROUPS)
    full = full.reshape(C, N)                           # [512, N]
    o = jax.lax.dot(proj_w.astype(bf16), full,
                    preferred_element_type=f32)         # [256, N]
    # return bf16: halves the D2H transfer over the slow tunnel; host
    # casts back to f32 (rounding ~0.4%, well within the error budget)
    return o.reshape(256, H, W).astype(bf16)


def _build_args(x, qkv_w, dw_w, proj_w, temperature, scale,
                crpe_w0, crpe_b0, crpe_w1, crpe_b1, crpe_w2, crpe_b2):
    x = np.asarray(x, dtype=np.float32)
    qkv_w = np.asarray(qkv_w, dtype=np.float32).reshape(3072, 512)
    dw_w = np.asarray(dw_w, dtype=np.float32).reshape(3072, 3, 3)
    proj_w = np.asarray(proj_w, dtype=np.float32).reshape(1024, 512)
    temperature = np.asarray(temperature, dtype=np.float32)
    scale = np.asarray(scale, dtype=np.float32)
    crpe_ws = [np.asarray(w, dtype=np.float32) for w in (crpe_w0, crpe_w1, crpe_w2)]
    crpe_bs = [np.asarray(b, dtype=np.float32) for b in (crpe_b0, crpe_b1, crpe_b2)]

    b = x.shape[0]
    B = b // P

    xs, qkvs, dws, crpews, crpebs, projs, temps, scales = ([] for _ in range(8))
    for core in range(NCORES):
        item = core // HG
        g = core % HG
        p = item // B                      # path of this batch item
        heads = list(range(g * HPG, (g + 1) * HPG))

        # qkv rows for path p: q block then k,v; within each, this group's heads
        base = p * 3 * C
        rows = []
        for sec in range(3):               # q, k, v sections
            lo = base + sec * C + g * HPG * CH
            rows.append(np.arange(lo, lo + HPG * CH))
        rows = np.concatenate(rows)
        qkvs.append(qkv_w[rows])
        dws.append(dw_w[rows])

        # crpe filters: pad every window to 7x7 (zero padding keeps conv exact)
        cw = np.zeros((HPG, CH, 7, 7), dtype=np.float32)
        cb = np.zeros((HPG,), dtype=np.float32)
        for j, h in enumerate(heads):
            wi, li = _head_window(h)
            hs = HEAD_SPLITS[wi]
            win = WINDOWS[wi]
            pad = (7 - win) // 2
            cw[j, :, pad:7 - pad, pad:7 - pad] = crpe_ws[wi][p * hs + li]
            cb[j] = crpe_bs[wi][p * hs + li]
        crpews.append(cw)
        crpebs.append(cb)

        projs.append(proj_w[p * C + g * 256: p * C + (g + 1) * 256])
        temps.append(temperature[p, heads])
        scales.append(scale[p, heads])
        # ship x as bf16: the device casts to bf16 before the qkv matmul
        # anyway, so this halves the dominant transfer at no numerical cost
        xs.append(x[item].astype(jnp.bfloat16))

    return [np.stack(a) for a in (xs, qkvs, dws, crpews, crpebs, projs, temps, scales)], b


def _assemble(outs, b):
    result = np.empty((b, C, H, W), dtype=np.float32)
    for core in range(NCORES):
        item, g = core // HG, core % HG
        result[item, g * 256:(g + 1) * 256] = np.asarray(outs[core], dtype=np.float32)
    return result


def kernel(**inputs):
    args, b = _build_args(**inputs)
    outs = np.asarray(_device_fn(*args))   # [8, 256, 128, 128]
    return _assemble(outs, b)
